# revision 38
# baseline (speedup 1.0000x reference)
"""Trainium2 Bass kernel for nn_ContentOnlyModel (embedding_lookup).

Model: score[b,t] = MLP(LN(txt_table[id]), LN(img_table[id])) — a pure
per-id function.  Host folds LN into the tables (row-wise, id-independent),
concatenates txt+img rows into one [V, 1280] fp16 table, and dedupes the
51200 requested ids.  The 8 cores are vocab-parallel: core k holds rows
[k*12501, (k+1)*12501) so dma_gather's int16 indices are in range.  Each
core gathers its unique ids with a transposing dma_gather (row value d
lands at partition d%128, chunk d//128 — exactly the matmul contraction
layout), then runs the 3-layer MLP on PE/ACT.  Host scatters the per-id
scores back to token positions, adds the final bias, and masks id==0.

Schedule (TimelineSim 47.7us, DMA-bound: 36.4us of gather transfers at the
360 B/ns model roofline):
  - 256-row gathers keep PE within one gather+sem of the DMA stream; the
    (128, 128) tail taper shortens the post-last-gather drain chain.
  - w1|w2|w3 ship as one packed f16 DMA; idx loads split (32 cols first)
    so the first desc-gen starts at ~3.0us.
  - ps3 PSUM ring of 2 (was 1) removes an m3->cp ladder that throttled
    both the steady state and the drain.
  - explicit no-sync deps push stale m3s behind the last unit's M1 so the
    PE in-order queue never head-blocks the critical drain chain.
  - single out DMA after the final copy (transfer is ~57ns; splitting only
    serializes on the shared HWDGE).

Dead ends (measured): int8/uint16/int16 matmul is rejected by the BIR
verifier (only f32/f16/bf16/fp8 lowers), fp8e4m3 tables miss the 2e-2
tolerance (~4e-2 propagated), swdge scatter-add prepare+trigger for the
output deadlocks Tile's DMASW lane accounting (Rust-side, unpatchable),
and an iota-indexed speculative prefix gather (pref=) loses ~1us: the
real stream is pinned at ~4.7us by the idx chain, the early DMA idle
already hides the weight loads, and the prefix only adds bytes.
"""

import sys

for _p in ("/opt/trn_rl_repo",):
    if _p not in sys.path:
        sys.path.insert(0, _p)

import numpy as np

import concourse.bacc as bacc
import concourse.bass_isa as bass_isa
import concourse.mybir as mybir
import concourse.tile as tile
from concourse.bass_utils import run_bass_kernel_spmd

# Tile assigns every Pool DMA a DMASW lane and expects its fixed +16 on the
# lane sem; a prepare_only scatter with a user `sem=` diverts that inc and
# deadlocks the epilogue lane waits.  Exempting the prep from the lanes (like
# the remote-DMA prep protocol) keeps lane accounting consistent; completion
# ordering is restored with an explicit gpsimd.wait_ge on the user sem.
if mybir.InstDMAScatterAddAnt not in getattr(
        bass_isa, "UserSyncedRemoteDMADescs").__args__         if hasattr(getattr(bass_isa, "UserSyncedRemoteDMADescs"), "__args__")         else True:
    pass

N_CORES = 8
I_FULL = 100001          # vocab rows
DT, DI = 768, 512        # txt/img dims
D_COMB = DT + DI         # 1280
NCH = D_COMB // 128      # 10 contraction chunks
HM, H = 64, 128
V8 = 12501               # rows per core shard (8*12501 = 100008 >= 100001)
CH = 512                 # ids per pipeline chunk
EPS = 1e-5

_nc_cache: dict[int, object] = {}
LABELS: dict[str, str] = {}  # instruction name -> semantic label (debug aid)


def _lab(inst, label):
    try:
        LABELS[inst.ins.name] = label
    except Exception:
        pass
    return inst


def build_nc(n_pad: int, ch: int = 256, xt_bufs: int = 4, h_bufs: int = 3,
             ps_bufs: int = 2, wstat: int = 3, nq: int = 1,
             scratch: int = 16384, strip: int = 256, lookahead: int = 2,
             tail: tuple = (128, 128), dep_cut: int = 3,
             m3lag: int = 2, idx_pool: bool = False, dve_tail: bool = False,
             pref: int = 0, split_last: bool = True):
    """Device program: gather n_pad ids from the local table shard and
    score them.  Shared by all 8 cores (SPMD).

    wstat>1 groups that many token-chunks per weight pass (weight-stationary
    over the group, fewer LDWEIGHTS)."""
    assert n_pad % 128 == 0
    n_tot = n_pad + pref
    f16, f32, i16 = mybir.dt.float16, mybir.dt.float32, mybir.dt.int16
    WCOLS = NCH * 128 + 128 + 8  # w1 | w2 | w3 packed column-wise

    nc = bacc.Bacc("TRN2", target_bir_lowering=False, debug=False,
                   num_devices=N_CORES, num_swdge_queues=nq,
                   dynamic_dma_scratch_size=scratch)
    table = nc.dram_tensor("table", [V8, D_COMB], f16, kind="ExternalInput")
    idxs = nc.dram_tensor("idxs", [128, n_pad // 16], i16, kind="ExternalInput")
    wpack = nc.dram_tensor("wpack", [128, WCOLS], f16, kind="ExternalInput")
    bias = nc.dram_tensor("bias", [128, 2], f32, kind="ExternalInput")
    out = nc.dram_tensor("out", [1, n_pad + pref], f32, kind="ExternalOutput")

    relu = mybir.ActivationFunctionType.Relu

    with tile.TileContext(nc) as tc:
        with (
            tc.tile_pool(name="const", bufs=1) as cpool,
            tc.tile_pool(name="x", bufs=xt_bufs) as xpool,
            tc.tile_pool(name="h", bufs=h_bufs) as hpool,
            tc.tile_pool(name="ps", bufs=ps_bufs, space="PSUM") as pspool,
            tc.tile_pool(name="ps1g", bufs=wstat + 1, space="PSUM") as ps1pool,
            tc.tile_pool(name="ob", bufs=1) as opool,
        ):
            wpack_t = cpool.tile([128, WCOLS], f16)
            w1_t = wpack_t[:, :NCH * 128]
            w2_t = wpack_t[:, NCH * 128:NCH * 128 + 128]
            w3_t = wpack_t[:, NCH * 128 + 128:]
            bias_t = cpool.tile([128, 2], f32)
            idx_t = cpool.tile([128, n_pad // 16], i16)
            first_cols = min(CH // 16, n_pad // 16)
            if idx_pool:
                nc.gpsimd.dma_start(out=idx_t[:, :first_cols],
                                    in_=idxs[:, :first_cols])
            else:
                nc.sync.dma_start(out=idx_t[:, :first_cols],
                                  in_=idxs[:, :first_cols])
            if n_pad // 16 > first_cols:
                nc.sync.dma_start(out=idx_t[:, first_cols:],
                                  in_=idxs[:, first_cols:])
            nc.sync.dma_start(out=wpack_t[:], in_=wpack[:])
            nc.sync.dma_start(out=bias_t[:], in_=bias[:])

            # PE warmup: dummy matmuls release the HAM clock gate during the
            # initial gather latency so real matmuls start at full clock.
            wu_rhs = cpool.tile([128, 512], f16)
            nc.vector.memset(wu_rhs[:], 0)
            wu_ps = pspool.tile([128, 512], f32, tag="ps2", name="wups")
            for _ in range(16):
                nc.tensor.matmul(wu_ps[:], lhsT=wu_rhs[:, :128],
                                 rhs=wu_rhs[:], start=True, stop=True)

            ob_all = opool.tile([1, n_tot], f32)

            # gather units: uniform ch-row gathers with a tapered tail so
            # the drain chain after the final bytes land is short.  compute
            # units: one per gather.
            tail_sum = sum(tail)
            if n_pad > tail_sum:
                body = n_pad - tail_sum
                g_sizes = ([ch] * (body // ch) + [128] * (body % ch // 128)
                           + list(tail))
            else:
                g_sizes = [128] * (n_pad // 128)
            if pref:
                # speculative prefix: gather shard rows [0, pref) with
                # device-generated indices — no idx-DMA dependency, so the
                # DMA stream starts ~2.7us earlier.  idx_t only covers the
                # real (post-prefix) gathers.
                assert pref % 128 == 0
                iota_t = cpool.tile([16, pref // 16], i16)
                nc.gpsimd.iota(iota_t[:], [[16, pref // 16]], base=0,
                               channel_multiplier=1)
                g_sizes = [pref] + g_sizes
            g_offs = [sum(g_sizes[:i]) for i in range(len(g_sizes))]
            n_g = len(g_sizes)
            c_units = []  # (gather_idx, col_offset, size)
            for gi in range(n_g):
                c_units.append((gi, 0, g_sizes[gi]))
            n_cu = len(c_units)
            users_left = {gi: sum(1 for g, _, _ in c_units if g == gi)
                          for gi in range(n_g)}

            xts, ps1s, h1s, ps2s, h2s, ps3s = {}, {}, {}, {}, {}, {}

            def gather(gi):
                gsz = g_sizes[gi]
                xt = xpool.tile([128, NCH, gsz], f16, tag="xt", name="xt")
                if pref and gi == 0:
                    iap = iota_t[:, :]
                else:
                    off = g_offs[gi] - (pref if pref else 0)
                    iap = idx_t[:, off // 16:(off + gsz) // 16]
                if split_last and gi == n_g - 1:
                    # split the final gather by dims: chunks 0-4 land one
                    # half-transfer earlier, so half of the last unit's M1
                    # hides under the second half's transfer + DMA sem.
                    half = NCH // 2
                    _lab(nc.gpsimd.dma_gather(
                        xt[:, :half, :], table[:, :half * 128], iap,
                        gsz, gsz, half * 128, elem_step=D_COMB,
                        transpose=True, queue_num=gi % nq), f"g({gi}a)")
                    _lab(nc.gpsimd.dma_gather(
                        xt[:, half:, :], table[:, half * 128:], iap,
                        gsz, gsz, D_COMB - half * 128, elem_step=D_COMB,
                        transpose=True, queue_num=gi % nq), f"g({gi}b)")
                else:
                    _lab(nc.gpsimd.dma_gather(
                        xt[:], table[:], iap,
                        gsz, gsz, D_COMB, transpose=True, queue_num=gi % nq),
                        f"g({gi})")
                xts[gi] = xt

            m1_last, m2_inst, m3_inst = {}, {}, {}

            def m1(cu):
                gi, co, sz = c_units[cu]
                ps1s[cu] = ps1pool.tile([128, sz], f32, tag="ps1", name="ps1")
                for c in range(NCH):
                    m1_last[cu] = _lab(nc.tensor.matmul(
                        ps1s[cu][:], lhsT=w1_t[:, c * 128:(c + 1) * 128],
                        rhs=xts[gi][:, c, co:co + sz],
                        start=(c == 0), stop=(c == NCH - 1)), f"m1({cu}).{c}")
                users_left[gi] -= 1
                if users_left[gi] == 0:
                    del xts[gi]

            a1_inst = {}

            def a1(cu):
                sz = c_units[cu][2]
                h1s[cu] = hpool.tile([128, sz], f16, tag="h1", name="h1")
                if dve_tail and cu == n_cu - 1:
                    a1_inst[cu] = _lab(nc.vector.tensor_scalar(
                        h1s[cu][:], ps1s[cu][:], bias_t[:, 0:1], 0.0,
                        mybir.AluOpType.add, mybir.AluOpType.max), f"a1({cu})")
                else:
                    a1_inst[cu] = _lab(nc.scalar.activation(
                        h1s[cu][:], ps1s[cu][:], relu,
                        bias=bias_t[:, 0:1]), f"a1({cu})")
                del ps1s[cu]

            def m2(cu):
                sz = c_units[cu][2]
                ps2s[cu] = pspool.tile([128, sz], f32, tag="ps2", name="ps2")
                m2_inst[cu] = _lab(nc.tensor.matmul(ps2s[cu][:], lhsT=w2_t[:],
                                 rhs=h1s[cu][:], start=True, stop=True), f"m2({cu})")
                if cu + 1 in m1_last and cu < n_cu - dep_cut:
                    tile.add_dep_helper(m2_inst[cu].ins, m1_last[cu + 1].ins,
                                        sync=False,
                                        reason="pipeline: M2_j after M1_j+1")
                del h1s[cu]

            def a2(cu):
                sz = c_units[cu][2]
                h2s[cu] = hpool.tile([128, sz], f16, tag="h2", name="h2")
                if dve_tail and cu == n_cu - 1:
                    _lab(nc.vector.tensor_scalar(
                        h2s[cu][:], ps2s[cu][:], bias_t[:, 1:2], 0.0,
                        mybir.AluOpType.add, mybir.AluOpType.max), f"a2({cu})")
                else:
                    _lab(nc.scalar.activation(h2s[cu][:], ps2s[cu][:], relu,
                                         bias=bias_t[:, 1:2]), f"a2({cu})")
                del ps2s[cu]

            def m3(cu):
                sz = c_units[cu][2]
                ps3s[cu] = pspool.tile([1, sz], f32, tag="ps3", name="ps3", bufs=2)
                inst = _lab(nc.tensor.matmul(ps3s[cu][:], lhsT=w3_t[:, 0:1],
                                 rhs=h2s[cu][:], start=True, stop=True), f"m3({cu})")
                m3_inst[cu] = inst
                if cu + 1 in m2_inst and cu < n_cu - dep_cut:
                    tile.add_dep_helper(inst.ins, m2_inst[cu + 1].ins, sync=False,
                                        reason="pipeline: M3_j after M2_j+1")
                del h2s[cu]

            def cp(cu):
                gi, co, sz = c_units[cu]
                off = g_offs[gi] + co
                _lab(nc.vector.tensor_copy(ob_all[:, off:off + sz],
                                           ps3s[cu][:]), f"cp({cu})")
                del ps3s[cu]

            issued = 0

            def issue_gathers(upto):
                nonlocal issued
                while issued < min(upto, n_g):
                    gather(issued)
                    issued += 1

            issue_gathers(lookahead)
            for j in range(n_cu + 1 + m3lag):
                if j < n_cu:
                    issue_gathers(c_units[j][0] + 1 + lookahead)
                    m1(j)
                if 0 <= j - 1 < n_cu:
                    m2(j - 1)
                if 0 <= j - m3lag < n_cu:
                    m3(j - m3lag)
                if j < n_cu:
                    a1(j)
                if 0 <= j - 1 < n_cu:
                    a2(j - 1)
                if 0 <= j - m3lag < n_cu:
                    cp(j - m3lag)

            # tail: stale m3s must not head-block the last unit's M1 in the
            # PE queue (their results are not on the critical path).
            for cu in range(max(0, n_cu - 5), n_cu - 1):
                if cu in m3_inst and (n_cu - 1) in m1_last:
                    tile.add_dep_helper(m3_inst[cu].ins, m1_last[n_cu - 1].ins,
                                        sync=False,
                                        reason="tail: M3_old after last M1")

            # single out DMA: the transfer is ~57ns, splitting would only
            # serialize on HWDGE.
            nc.sync.dma_start(out=out[0:1, :], in_=ob_all[:, :])

    nc.compile()
    return nc


def _prep_host(inputs):
    """Fold LN + layer1 layout on host; returns (comb_table_f16, weight
    arrays)."""
    txt = np.asarray(inputs["txt_table"], np.float32)
    img = np.asarray(inputs["img_table"], np.float32)

    def ln(x, g, b):
        mu = x.mean(axis=1, keepdims=True)
        xc = x - mu
        var = (xc * xc).mean(axis=1, keepdims=True)
        return xc * (1.0 / np.sqrt(var + EPS)) * g + b

    txt_n = ln(txt, np.asarray(inputs["ln_txt_g"], np.float32),
               np.asarray(inputs["ln_txt_b"], np.float32))
    img_n = ln(img, np.asarray(inputs["ln_img_g"], np.float32),
               np.asarray(inputs["ln_img_b"], np.float32))

    comb = np.zeros((N_CORES * V8, D_COMB), np.float16)
    comb[:I_FULL, :DT] = txt_n
    comb[:I_FULL, DT:] = img_n

    # lhsT layer1: [d_in_chunk(128 part), chunk, h] ; block diagonal
    txt_w = np.asarray(inputs["txt_w"], np.float32)   # [64, 768]
    img_w = np.asarray(inputs["img_w"], np.float32)   # [64, 512]
    w_comb = np.zeros((D_COMB, H), np.float32)
    w_comb[:DT, :HM] = txt_w.T
    w_comb[DT:, HM:] = img_w.T
    w1_dram = np.ascontiguousarray(
        w_comb.reshape(NCH, 128, H).transpose(1, 0, 2)).astype(np.float16)

    w2_dram = np.asarray(inputs["fus_w1"], np.float32).T.astype(np.float16)
    w3_dram = np.zeros((128, 8), np.float16)
    w3_dram[:, 0] = np.asarray(inputs["fus_w2"], np.float32)[0]
    wpack_dram = np.ascontiguousarray(np.concatenate(
        [w1_dram.reshape(128, NCH * 128), w2_dram, w3_dram], axis=1))
    bias_dram = np.zeros((128, 2), np.float32)
    bias_dram[:, 0] = np.concatenate([
        np.asarray(inputs["txt_bias"], np.float32),
        np.asarray(inputs["img_bias"], np.float32),
    ])
    bias_dram[:, 1] = np.asarray(inputs["fus_b1"], np.float32)
    return comb, wpack_dram, bias_dram


def _wrap_idxs(local: np.ndarray, n_pad: int) -> np.ndarray:
    """idx i -> partition i%16, column i//16; replicated to 128 partitions."""
    padded = np.zeros(n_pad, np.int16)
    padded[:len(local)] = local
    tile16 = padded.reshape(n_pad // 16, 16).T  # [16, n_pad//16]
    return np.ascontiguousarray(np.tile(tile16, (8, 1)))


def kernel(**inputs):
    pos = np.asarray(inputs["pos_seqs"])
    neg = np.asarray(inputs["neg_seqs"])
    B, T = pos.shape

    comb, wpack_dram, bias_dram = _prep_host(inputs)

    ids_all = np.concatenate([pos.ravel(), neg.ravel()]).astype(np.int64)
    uniq, inv = np.unique(ids_all, return_inverse=True)
    bounds = np.searchsorted(uniq, np.arange(1, N_CORES) * V8)
    segs = np.split(uniq, bounds)
    counts = [len(s) for s in segs]
    n_pad = max(CH, -(-max(counts) // CH) * CH)

    in_maps = []
    for k in range(N_CORES):
        local = (segs[k] - k * V8).astype(np.int16)
        in_maps.append({
            "table": np.ascontiguousarray(comb[k * V8:(k + 1) * V8]),
            "idxs": _wrap_idxs(local, n_pad),
            "wpack": wpack_dram,
            "bias": bias_dram,
        })

    nc = _nc_cache.get(n_pad)
    if nc is None:
        nc = build_nc(n_pad)
        _nc_cache[n_pad] = nc

    res = None
    for attempt in range(3):
        try:
            res = run_bass_kernel_spmd(nc, in_maps,
                                       core_ids=list(range(N_CORES)))
            # materialize inside the retry: device errors can surface lazily
            outs = [np.asarray(res.results[k]["out"]) for k in range(N_CORES)]
            break
        except Exception:
            # transient NRT_EXEC_UNIT_UNRECOVERABLE has been observed on the
            # axon workers; a clean retry succeeds
            if attempt == 2:
                raise
            import time
            time.sleep(5)
            try:
                import jax
                jax.clear_backends()
            except Exception:
                pass

    score_uniq = np.concatenate(
        [outs[k][0, :counts[k]] for k in range(N_CORES)])
    fus_b2 = float(np.asarray(inputs["fus_b2"], np.float32)[0])
    scores = score_uniq[inv].astype(np.float32) + fus_b2
    scores[ids_all == 0] = 0.0
    n_tok = B * T
    pos_out = scores[:n_tok].reshape(B, T)
    neg_out = scores[n_tok:].reshape(B, T)
    return pos_out, neg_out

